# revision 11
# baseline (speedup 1.0000x reference)
"""Trainium2 Bass kernel for packed-sequence attention (nn_Attention).

Sharding (8 cores): core c handles sequence c//2 and head-group c%2
(8 of 16 heads).  Each core runs an identical SPMD program:
  A) QKV projection (x.T tiles @ wqkv.T column slices, bf16, f32 PSUM)
  B) fused RMSNorm (ones-matmul cross-partition sumsq) + RoPE
     (pair-swap via SBUF-SBUF DMA in a deinterleaved d-basis that is
     baked into the host-permuted wqkv rows)
  C) attention per head: S=K'Q panels -> exp on ACT -> PV + ones-row-sum
     matmuls accumulating in PSUM; normalize O by the row sums
  D) wo matmul over this core's 8 head-chunks -> partial [L, 2048]
Host: pairs of cores holding the same sequence have complementary head
groups; their partial outputs are summed (row-parallel TP unshard).
"""

import math
import numpy as np
import ml_dtypes
from contextlib import ExitStack

P = 128
HD = 128
BF = ml_dtypes.bfloat16


# ---------------------------------------------------------------------------
# walrus workaround: this container's walrus accepts only ONE sync-wait per
# CTRL instruction, but TileContext's kernel-tail drain carries every
# outstanding wait -> 'Too many sync wait commands'.  Split them across a
# chain of drains.
_PATCHED = False


def _patch_tile_drain():
    global _PATCHED
    if _PATCHED:
        return
    import concourse.tile as _tile
    from concourse.vector_clock import ScopedClock as _ScopedClock

    def _drain_and_barrier(self, tick_clock, wait_clock):
        nc = self.nc
        drain_inst = nc.sync.drain()
        wait_clock.add_sem_waits(
            drain_inst.ins, _ScopedClock({None: tick_clock.global_clock})
        )
        si = drain_inst.ins.sync_info
        waits = list(si.on_wait or []) if si is not None else []
        if len(waits) > 1:
            si.on_wait = [waits[0]]
            for w in waits[1:]:
                d2 = nc.sync.drain()
                si2 = d2.ins.sync_info
                if si2 is None:
                    import bass_rust

                    d2.ins.sync_info = bass_rust.SyncInfo(on_wait=[w], on_update=[])
                else:
                    si2.on_wait = [w]
        nc.all_engine_barrier()
        assert self.sems is not None
        popped = nc._tile_sem_poison_stack.pop()
        assert popped is self._sem_poison
        nc.clear_and_free_semaphores(list(self.sems.allocated().values()))
        nc.all_engine_barrier()

    _tile.TileContext._drain_and_barrier = _drain_and_barrier
    _PATCHED = True


# ---------------------------------------------------------------------------


def _build_program(L, C, NP, DOUT, n_cores):
    """Build the SPMD per-core program.

    L: tokens per core (sequence length), C: model/contraction dim,
    NP: local head pairs (local heads = 2*NP), DOUT: wo output dim.
    """
    import concourse.bass as bass
    import concourse.mybir as mybir
    import concourse.tile as tile
    from concourse import bacc

    dt = mybir.dt
    AF = mybir.ActivationFunctionType
    OP = mybir.AluOpType

    NHL = 2 * NP
    TP = L // 512          # t/q panels
    KC = L // P            # key chunks / t subtiles
    CCH = C // P           # contraction chunks
    JP = DOUT // 512       # output column panels
    scale = 1.0 / math.sqrt(HD)
    EPS = 1e-5

    nc = bacc.Bacc("TRN2", target_bir_lowering=False, debug=False,
                   num_devices=n_cores)

    xT_d = nc.dram_tensor("xT", [C, L], dt.bfloat16, kind="ExternalInput").ap()
    wT_d = nc.dram_tensor("wT", [C, NP * 768], dt.bfloat16, kind="ExternalInput").ap()
    woT_d = nc.dram_tensor("woT", [NHL * HD, DOUT], dt.bfloat16, kind="ExternalInput").ap()
    cosT_d = nc.dram_tensor("cosT", [P, L], dt.bfloat16, kind="ExternalInput").ap()
    sinT_d = nc.dram_tensor("sinT", [P, L], dt.bfloat16, kind="ExternalInput").ap()
    qnw_d = nc.dram_tensor("qnw", [P, 1], dt.float32, kind="ExternalInput").ap()
    knw_d = nc.dram_tensor("knw", [P, 1], dt.float32, kind="ExternalInput").ap()
    out_d = nc.dram_tensor("out", [L, DOUT], dt.float32, kind="ExternalOutput").ap()

    with tile.TileContext(nc) as tc:
        with ExitStack() as ctx:
            const = ctx.enter_context(tc.tile_pool(name="const", bufs=1))
            ps = ctx.enter_context(tc.tile_pool(name="ps", bufs=4, space="PSUM"))
            stream = ctx.enter_context(tc.tile_pool(name="stream", bufs=2))
            qkv = ctx.enter_context(tc.tile_pool(name="qkv", bufs=4))
            opool = ctx.enter_context(tc.tile_pool(name="opool", bufs=NHL))
            work = ctx.enter_context(tc.tile_pool(name="work", bufs=3))
            dpool = ctx.enter_context(tc.tile_pool(name="dpool", bufs=4, space="DRAM"))

            cos_sb = const.tile([P, L], dt.bfloat16, tag="cos", bufs=1)
            nc.sync.dma_start(cos_sb[:], cosT_d[:])
            sin_sb = const.tile([P, L], dt.bfloat16, tag="sin", bufs=1)
            nc.sync.dma_start(sin_sb[:], sinT_d[:])
            qnw_sb = const.tile([P, 1], dt.float32, tag="qnw", bufs=1)
            nc.sync.dma_start(qnw_sb[:], qnw_d[:])
            knw_sb = const.tile([P, 1], dt.float32, tag="knw", bufs=1)
            nc.sync.dma_start(knw_sb[:], knw_d[:])
            ones_sb = const.tile([P, 1], dt.bfloat16, tag="ones", bufs=1)
            nc.vector.memset(ones_sb[:], 1.0)
            eps_sb = const.tile([P, 1], dt.float32, tag="eps", bufs=1)
            nc.vector.memset(eps_sb[:], EPS)

            o_tiles = []
            pair_state = {}

            def emit_proj(p):
                # -------- pair-level weight loads (one big DMA each) --------
                wqk_sb = stream.tile([P, CCH, 512], dt.bfloat16, tag="wqk",
                                     bufs=1, name=f"wqk{p}")
                nc.sync.dma_start(
                    wqk_sb[:],
                    wT_d[:, p * 768:p * 768 + 512].rearrange(
                        "(cc q) w -> q cc w", q=P))
                wv_sb = stream.tile([P, CCH, 256], dt.bfloat16, tag="wv",
                                    bufs=2, name=f"wv{p}")
                nc.sync.dma_start(
                    wv_sb[:],
                    wT_d[:, p * 768 + 512:p * 768 + 768].rearrange(
                        "(cc q) w -> q cc w", q=P))

                qr_t = [qkv.tile([P, L], dt.bfloat16, tag="qr", bufs=4,
                                 name=f"qr{p}_{i}") for i in range(2)]
                kr_t = [qkv.tile([P, L], dt.bfloat16, tag="kr", bufs=4,
                                 name=f"kr{p}_{i}") for i in range(2)]
                vv = qkv.tile([P, KC, 256], dt.bfloat16, tag="vv", bufs=2,
                              name=f"vv{p}")

                for tp in range(TP):
                    tsl = bass.ts(tp, 512)
                    xtc = stream.tile([P, CCH, 512], dt.bfloat16, tag="xtc",
                                      bufs=2, name=f"xtc{p}_{tp}")
                    nc.sync.dma_start(
                        xtc[:],
                        xT_d[:, tsl].rearrange("(cc q) w -> q cc w", q=P))

                    # ---- q/k projection, two units at a time (2 live psums) ----
                    for grp in range(2):
                        pq2 = [ps.tile([P, 512], dt.float32, tag="acc", bufs=4,
                                       name=f"pqk{grp}_{i}") for i in range(2)]
                        for cc in range(CCH):
                            for i in range(2):
                                u = grp * 2 + i
                                nc.tensor.matmul(
                                    pq2[i][:],
                                    wqk_sb[:, cc, u * 128:(u + 1) * 128],
                                    xtc[:, cc, :],
                                    start=(cc == 0), stop=(cc == CCH - 1))
                        # fused rmsnorm + rope (q0 q1 | k0 k1)
                        for i in range(2):
                            u = grp * 2 + i
                            wnorm = qnw_sb if u < 2 else knw_sb
                            dest = qr_t[u % 2] if u < 2 else kr_t[u % 2]
                            qraw = work.tile([P, 512], dt.float32, tag="qraw",
                                             bufs=3)
                            nc.vector.tensor_copy(qraw[:], pq2[i][:])
                            q2 = work.tile([P, 512], dt.bfloat16, tag="q2", bufs=2)
                            nc.vector.tensor_mul(q2[:], qraw[:], qraw[:])
                            ssq = ps.tile([1, 512], dt.float32, tag="s", bufs=4)
                            nc.tensor.matmul(ssq[:], ones_sb[:], q2[:],
                                             start=True, stop=True)
                            rms = work.tile([1, 512], dt.float32, tag="rms", bufs=2)
                            nc.scalar.activation(rms[:], ssq[:], AF.Sqrt,
                                                 bias=eps_sb[0:1, :],
                                                 scale=1.0 / HD)
                            rs = work.tile([1, 512], dt.float32, tag="rs", bufs=2)
                            nc.vector.reciprocal_approx_fast(rs[:], rms[:])
                            rsd = dpool.tile([1, 512], dt.float32, tag="rsd",
                                             bufs=4)
                            nc.gpsimd.dma_start(rsd[:], rs[:])
                            rsb = work.tile([P, 512], dt.float32, tag="rsb",
                                            bufs=2)
                            nc.gpsimd.dma_start(rsb[:],
                                                rsd[:].to_broadcast((P, 512)))
                            qs = work.tile([P, 512], dt.bfloat16, tag="qs", bufs=3)
                            nc.vector.scalar_tensor_tensor(
                                qs[:], qraw[:], wnorm[:], rsb[:],
                                op0=OP.mult, op1=OP.mult)
                            qsw = work.tile([P, 512], dt.bfloat16, tag="qsw",
                                            bufs=3)
                            nc.gpsimd.dma_start(qsw[0:64, :], qs[64:128, :])
                            nc.gpsimd.dma_start(qsw[64:128, :], qs[0:64, :])
                            t1 = work.tile([P, 512], dt.bfloat16, tag="t1", bufs=2)
                            nc.vector.tensor_mul(t1[:], qs[:], cos_sb[:, tsl])
                            t2 = work.tile([P, 512], dt.bfloat16, tag="t2", bufs=2)
                            nc.vector.tensor_mul(t2[:], qsw[:], sin_sb[:, tsl])
                            nc.vector.tensor_add(dest[:, tsl], t1[:], t2[:])

                    # ---- v projection, two subtiles at a time ----
                    for grp in range(2):
                        pv2 = [ps.tile([P, 256], dt.float32, tag="acc", bufs=4,
                                       name=f"pv{grp}_{i}") for i in range(2)]
                        for cc in range(CCH):
                            for i in range(2):
                                ts_ = grp * 2 + i
                                nc.tensor.matmul(
                                    pv2[i][:],
                                    xtc[:, cc, ts_ * 128:(ts_ + 1) * 128],
                                    wv_sb[:, cc, :],
                                    start=(cc == 0), stop=(cc == CCH - 1))
                        for i in range(2):
                            ts_ = grp * 2 + i
                            nc.vector.tensor_copy(vv[:, tp * 4 + ts_, :],
                                                  pv2[i][:])

                pair_state[p] = (qr_t, kr_t, vv)

            def emit_attn(p):
                qr_t, kr_t, vv = pair_state.pop(p)
                o_pair = [opool.tile([P, L], dt.bfloat16, tag="o", bufs=NHL,
                                     name=f"o{p}_{i}") for i in range(2)]
                o_tiles.extend(o_pair)
                for qp in range(TP):
                    qsl = bass.ts(qp, 512)
                    po = [ps.tile([P, 512], dt.float32, tag="acc", bufs=4,
                                  name=f"po{i}") for i in range(2)]
                    prs = [ps.tile([1, 512], dt.float32, tag="s", bufs=4,
                                   name=f"prs{i}") for i in range(2)]
                    for kc in range(KC):
                        for l in range(2):
                            pss = ps.tile([P, 512], dt.float32, tag="s", bufs=4)
                            nc.tensor.matmul(
                                pss[:], kr_t[l][:, kc * P:(kc + 1) * P],
                                qr_t[l][:, qsl], start=True, stop=True)
                            e = work.tile([P, 512], dt.bfloat16, tag="e", bufs=4)
                            nc.scalar.activation(e[:], pss[:], AF.Exp,
                                                 scale=scale)
                            nc.tensor.matmul(
                                po[l][:], vv[:, kc, l * 128:(l + 1) * 128], e[:],
                                start=(kc == 0), stop=(kc == KC - 1))
                            nc.tensor.matmul(
                                prs[l][:], ones_sb[:], e[:],
                                start=(kc == 0), stop=(kc == KC - 1))
                    for l in range(2):
                        rr = work.tile([1, 512], dt.float32, tag="rs", bufs=2)
                        nc.vector.reciprocal_approx_fast(rr[:], prs[l][:])
                        rrd = dpool.tile([1, 512], dt.float32, tag="rsd", bufs=4)
                        nc.gpsimd.dma_start(rrd[:], rr[:])
                        rrb = work.tile([P, 512], dt.float32, tag="rsb", bufs=2)
                        nc.gpsimd.dma_start(rrb[:], rrd[:].to_broadcast((P, 512)))
                        nc.vector.tensor_mul(o_pair[l][:, qsl], po[l][:], rrb[:])

            # software-pipeline: proj(p) overlaps attn(p-1)
            for p in range(NP):
                emit_proj(p)
                if p > 0:
                    emit_attn(p - 1)
            emit_attn(NP - 1)

            # ---------------- wo ----------------
            for jp in range(JP):
                jsl = bass.ts(jp, 512)
                wo_jp = stream.tile([P, NHL, 512], dt.bfloat16, tag="wo", bufs=2)
                nc.sync.dma_start(
                    wo_jp[:], woT_d[:, jsl].rearrange("(h q) j -> q h j", q=P))
                for tt in range(L // P):
                    pw = ps.tile([P, 512], dt.float32, tag="acc", bufs=4)
                    for h in range(NHL):
                        nc.tensor.matmul(
                            pw[:], o_tiles[h][:, tt * P:(tt + 1) * P],
                            wo_jp[:, h, :], start=(h == 0), stop=(h == NHL - 1))
                    osb = work.tile([P, 512], dt.float32, tag="outsb", bufs=2)
                    nc.vector.tensor_copy(osb[:], pw[:])
                    nc.sync.dma_start(out_d[tt * P:(tt + 1) * P, jsl], osb[:])

    nc.compile()
    return nc


def _host_prepare(x, rope_cos, rope_sin, wqkv, wo, q_norm_w, k_norm_w,
                  L, C, NP, DOUT, n_cores):
    """Build per-core input dicts."""
    NH_TOT = wqkv.shape[0] // 3 // HD
    NHL = 2 * NP
    n_seq = n_cores // 2
    perm = np.concatenate([np.arange(0, HD, 2), np.arange(1, HD, 2)])  # deinterleave

    qn_p = np.ascontiguousarray(q_norm_w[perm].reshape(HD, 1)).astype(np.float32)
    kn_p = np.ascontiguousarray(k_norm_w[perm].reshape(HD, 1)).astype(np.float32)

    wq = wqkv[0 * NH_TOT * HD:1 * NH_TOT * HD].reshape(NH_TOT, HD, C)
    wk = wqkv[1 * NH_TOT * HD:2 * NH_TOT * HD].reshape(NH_TOT, HD, C)
    wv = wqkv[2 * NH_TOT * HD:3 * NH_TOT * HD].reshape(NH_TOT, HD, C)

    in_maps = []
    for c in range(n_cores):
        b = c // 2
        hg = c % 2
        heads = list(range(hg * NHL, hg * NHL + NHL))
        xb = x[b * L:(b + 1) * L]                      # [L, C]
        xT = np.ascontiguousarray(xb.T).astype(BF)      # [C, L]

        blocks = []
        for pidx in range(NP):
            h0, h1 = heads[2 * pidx], heads[2 * pidx + 1]
            blocks += [wq[h0][perm], wq[h1][perm],
                       wk[h0][perm], wk[h1][perm],
                       wv[h0], wv[h1]]
        wT = np.ascontiguousarray(np.concatenate(blocks, axis=0).T).astype(BF)

        woT_rows = wo[:, heads[0] * HD:(heads[-1] + 1) * HD].T  # [NHL*HD, DOUT]
        woT = np.ascontiguousarray(woT_rows).astype(BF)

        cosb = rope_cos[b * L:(b + 1) * L].T            # [64, L]
        sinb = rope_sin[b * L:(b + 1) * L].T
        cosT = np.ascontiguousarray(np.concatenate([cosb, cosb], 0)).astype(BF)
        sinT = np.ascontiguousarray(np.concatenate([-sinb, sinb], 0)).astype(BF)

        in_maps.append({
            "xT": xT, "wT": wT, "woT": woT, "cosT": cosT, "sinT": sinT,
            "qnw": qn_p, "knw": kn_p,
        })
    return in_maps


def _reference_numpy(x, rope_cos, rope_sin, cu, max_length,
                     wqkv, wo, q_norm_w, k_norm_w):
    """Pure-numpy fallback (exact reference math) for non-uniform cu."""
    T, dim = x.shape
    nh = dim // HD
    qkv = (x @ wqkv.T).reshape(T, 3, nh, HD)
    q, k, v = qkv[:, 0], qkv[:, 1], qkv[:, 2]

    def rmsnorm(t, w):
        return t / np.sqrt((t * t).mean(-1, keepdims=True) + 1e-5) * w

    def rope(t):
        tr = t.reshape(t.shape[:-1] + (HD // 2, 2))
        e, o = tr[..., 0], tr[..., 1]
        cc = rope_cos[:, None, :]
        ss = rope_sin[:, None, :]
        return np.stack([e * cc - o * ss, e * ss + o * cc], -1).reshape(t.shape)

    q = rope(rmsnorm(q, q_norm_w))
    k = rope(rmsnorm(k, k_norm_w))
    o = np.zeros((T, nh, HD), np.float32)
    nb = len(cu) - 1
    for i in range(nb):
        s, e_ = int(cu[i]), int(cu[i + 1])
        if e_ <= s:
            continue
        qs_, ks_, vs_ = q[s:e_], k[s:e_], v[s:e_]
        sc = np.einsum("lhd,mhd->hlm", qs_, ks_) / math.sqrt(HD)
        sc = sc - sc.max(-1, keepdims=True)
        a = np.exp(sc)
        a /= a.sum(-1, keepdims=True)
        o[s:e_] = np.einsum("hlm,mhd->lhd", a, vs_)
    return (o.reshape(T, dim) @ wo.T).astype(np.float32)


def kernel(x, rope_cos, rope_sin, cu, max_length, wqkv, wo, q_norm_w, k_norm_w):
    x = np.asarray(x, np.float32)
    rope_cos = np.asarray(rope_cos, np.float32)
    rope_sin = np.asarray(rope_sin, np.float32)
    cu = np.asarray(cu)
    wqkv = np.asarray(wqkv, np.float32)
    wo = np.asarray(wo, np.float32)
    q_norm_w = np.asarray(q_norm_w, np.float32)
    k_norm_w = np.asarray(k_norm_w, np.float32)

    T, C = x.shape
    N_CORES = 8
    L = T // 4
    expect_cu = np.arange(5) * L
    if (len(cu) != 5 or not np.array_equal(np.asarray(cu).ravel(), expect_cu)
            or T % 4 != 0 or L % 512 != 0 or C % P != 0):
        return _reference_numpy(x, rope_cos, rope_sin, cu, max_length,
                                wqkv, wo, q_norm_w, k_norm_w)

    NP = (C // HD) // 2 // 2          # local head pairs = NH/2/2
    DOUT = wo.shape[0]

    from concourse.bass_utils import run_bass_kernel_spmd

    nc = _build_program(L, C, NP, DOUT, N_CORES)
    in_maps = _host_prepare(x, rope_cos, rope_sin, wqkv, wo, q_norm_w, k_norm_w,
                            L, C, NP, DOUT, N_CORES)
    res = run_bass_kernel_spmd(nc, in_maps, list(range(N_CORES)))

    out = np.empty((T, DOUT), np.float32)
    for b in range(4):
        out[b * L:(b + 1) * L] = (res.results[2 * b]["out"]
                                  + res.results[2 * b + 1]["out"])
    return out


# revision 12
# speedup vs baseline: 1.0760x; 1.0760x over previous
"""Trainium2 Bass kernel for packed-sequence attention (nn_Attention).

Sharding (8 cores): core c handles sequence c//2 and head-group c%2
(8 of 16 heads).  Each core runs an identical SPMD program:
  A) QKV projection (x.T tiles @ wqkv.T column slices, bf16, f32 PSUM)
  B) fused RMSNorm (ones-matmul cross-partition sumsq) + RoPE
     (pair-swap via SBUF-SBUF DMA in a deinterleaved d-basis that is
     baked into the host-permuted wqkv rows)
  C) attention per head: S=K'Q panels -> exp on ACT -> PV + ones-row-sum
     matmuls accumulating in PSUM; normalize O by the row sums
  D) wo matmul over this core's 8 head-chunks -> partial [L, 2048]
Host: pairs of cores holding the same sequence have complementary head
groups; their partial outputs are summed (row-parallel TP unshard).
"""

import math
import numpy as np
import ml_dtypes
from contextlib import ExitStack

P = 128
HD = 128
BF = ml_dtypes.bfloat16


# ---------------------------------------------------------------------------
# walrus workaround: this container's walrus accepts only ONE sync-wait per
# CTRL instruction, but TileContext's kernel-tail drain carries every
# outstanding wait -> 'Too many sync wait commands'.  Split them across a
# chain of drains.
_PATCHED = False


def _patch_tile_drain():
    global _PATCHED
    if _PATCHED:
        return
    import concourse.tile as _tile
    from concourse.vector_clock import ScopedClock as _ScopedClock

    def _drain_and_barrier(self, tick_clock, wait_clock):
        nc = self.nc
        drain_inst = nc.sync.drain()
        wait_clock.add_sem_waits(
            drain_inst.ins, _ScopedClock({None: tick_clock.global_clock})
        )
        si = drain_inst.ins.sync_info
        waits = list(si.on_wait or []) if si is not None else []
        if len(waits) > 1:
            si.on_wait = [waits[0]]
            for w in waits[1:]:
                d2 = nc.sync.drain()
                si2 = d2.ins.sync_info
                if si2 is None:
                    import bass_rust

                    d2.ins.sync_info = bass_rust.SyncInfo(on_wait=[w], on_update=[])
                else:
                    si2.on_wait = [w]
        nc.all_engine_barrier()
        assert self.sems is not None
        popped = nc._tile_sem_poison_stack.pop()
        assert popped is self._sem_poison
        nc.clear_and_free_semaphores(list(self.sems.allocated().values()))
        nc.all_engine_barrier()

    _tile.TileContext._drain_and_barrier = _drain_and_barrier
    _PATCHED = True


# ---------------------------------------------------------------------------


def _build_program(L, C, NP, DOUT, n_cores):
    """Build the SPMD per-core program.

    L: tokens per core (sequence length), C: model/contraction dim,
    NP: local head pairs (local heads = 2*NP), DOUT: wo output dim.
    """
    import concourse.bass as bass
    import concourse.mybir as mybir
    import concourse.tile as tile
    from concourse import bacc

    dt = mybir.dt
    AF = mybir.ActivationFunctionType
    OP = mybir.AluOpType

    NHL = 2 * NP
    TP = L // 512          # t/q panels
    KC = L // P            # key chunks / t subtiles
    CCH = C // P           # contraction chunks
    JP = DOUT // 512       # output column panels
    scale = 1.0 / math.sqrt(HD)
    EPS = 1e-5

    nc = bacc.Bacc("TRN2", target_bir_lowering=False, debug=False,
                   num_devices=n_cores)

    xT_d = nc.dram_tensor("xT", [C, L], dt.bfloat16, kind="ExternalInput").ap()
    wT_d = nc.dram_tensor("wT", [C, NP * 768], dt.bfloat16, kind="ExternalInput").ap()
    woT_d = nc.dram_tensor("woT", [NHL * HD, DOUT], dt.bfloat16, kind="ExternalInput").ap()
    cosT_d = nc.dram_tensor("cosT", [P, L], dt.bfloat16, kind="ExternalInput").ap()
    sinT_d = nc.dram_tensor("sinT", [P, L], dt.bfloat16, kind="ExternalInput").ap()
    qnw_d = nc.dram_tensor("qnw", [P, 1], dt.float32, kind="ExternalInput").ap()
    knw_d = nc.dram_tensor("knw", [P, 1], dt.float32, kind="ExternalInput").ap()
    out_d = nc.dram_tensor("out", [L, DOUT], dt.float32, kind="ExternalOutput").ap()

    with tile.TileContext(nc) as tc:
        with ExitStack() as ctx:
            const = ctx.enter_context(tc.tile_pool(name="const", bufs=1))
            ps = ctx.enter_context(tc.tile_pool(name="ps", bufs=4, space="PSUM"))
            stream = ctx.enter_context(tc.tile_pool(name="stream", bufs=2))
            qkv = ctx.enter_context(tc.tile_pool(name="qkv", bufs=4))
            opool = ctx.enter_context(tc.tile_pool(name="opool", bufs=NHL))
            work = ctx.enter_context(tc.tile_pool(name="work", bufs=3))
            dpool = ctx.enter_context(tc.tile_pool(name="dpool", bufs=4, space="DRAM"))

            cos_sb = const.tile([P, L], dt.bfloat16, tag="cos", bufs=1)
            nc.sync.dma_start(cos_sb[:], cosT_d[:])
            sin_sb = const.tile([P, L], dt.bfloat16, tag="sin", bufs=1)
            nc.sync.dma_start(sin_sb[:], sinT_d[:])
            qnw_sb = const.tile([P, 1], dt.float32, tag="qnw", bufs=1)
            nc.sync.dma_start(qnw_sb[:], qnw_d[:])
            knw_sb = const.tile([P, 1], dt.float32, tag="knw", bufs=1)
            nc.sync.dma_start(knw_sb[:], knw_d[:])
            ones_sb = const.tile([P, 1], dt.bfloat16, tag="ones", bufs=1)
            nc.vector.memset(ones_sb[:], 1.0)
            eps_sb = const.tile([P, 1], dt.float32, tag="eps", bufs=1)
            nc.vector.memset(eps_sb[:], EPS)

            o_tiles = []
            pair_state = {}

            def emit_proj(p):
                # -------- pair-level weight loads (one big DMA each) --------
                wqk_sb = stream.tile([P, CCH, 512], dt.bfloat16, tag="wqk",
                                     bufs=1, name=f"wqk{p}")
                wqsrc = wT_d[:, p * 768:p * 768 + 512].rearrange(
                    "(cc q) w -> q cc w", q=P)
                for sp in range(8):
                    c0, c1 = sp * CCH // 8, (sp + 1) * CCH // 8
                    nc.sync.dma_start(wqk_sb[:, c0:c1, :], wqsrc[:, c0:c1, :])
                wv_sb = stream.tile([P, CCH, 256], dt.bfloat16, tag="wv",
                                    bufs=2, name=f"wv{p}")
                wvsrc = wT_d[:, p * 768 + 512:p * 768 + 768].rearrange(
                    "(cc q) w -> q cc w", q=P)
                for sp in range(4):
                    c0, c1 = sp * CCH // 4, (sp + 1) * CCH // 4
                    nc.sync.dma_start(wv_sb[:, c0:c1, :], wvsrc[:, c0:c1, :])

                qr_t = [qkv.tile([P, L], dt.bfloat16, tag="qr", bufs=4,
                                 name=f"qr{p}_{i}") for i in range(2)]
                kr_t = [qkv.tile([P, L], dt.bfloat16, tag="kr", bufs=4,
                                 name=f"kr{p}_{i}") for i in range(2)]
                vv = qkv.tile([P, KC, 256], dt.bfloat16, tag="vv", bufs=2,
                              name=f"vv{p}")

                for tp in range(TP):
                    tsl = bass.ts(tp, 512)
                    xtc = stream.tile([P, CCH, 512], dt.bfloat16, tag="xtc",
                                      bufs=2, name=f"xtc{p}_{tp}")
                    xsrc = xT_d[:, tsl].rearrange("(cc q) w -> q cc w", q=P)
                    for sp in range(8):
                        c0, c1 = sp * CCH // 8, (sp + 1) * CCH // 8
                        nc.sync.dma_start(xtc[:, c0:c1, :], xsrc[:, c0:c1, :])

                    # ---- q/k projection, two units at a time (2 live psums) ----
                    for grp in range(2):
                        pq2 = [ps.tile([P, 512], dt.float32, tag="acc", bufs=4,
                                       name=f"pqk{grp}_{i}") for i in range(2)]
                        for cc in range(CCH):
                            for i in range(2):
                                u = grp * 2 + i
                                nc.tensor.matmul(
                                    pq2[i][:],
                                    wqk_sb[:, cc, u * 128:(u + 1) * 128],
                                    xtc[:, cc, :],
                                    start=(cc == 0), stop=(cc == CCH - 1))
                        # fused rmsnorm + rope (q0 q1 | k0 k1)
                        for i in range(2):
                            u = grp * 2 + i
                            wnorm = qnw_sb if u < 2 else knw_sb
                            dest = qr_t[u % 2] if u < 2 else kr_t[u % 2]
                            qraw = work.tile([P, 512], dt.float32, tag="qraw",
                                             bufs=3)
                            nc.vector.tensor_copy(qraw[:], pq2[i][:])
                            q2 = work.tile([P, 512], dt.bfloat16, tag="q2", bufs=2)
                            nc.vector.tensor_mul(q2[:], qraw[:], qraw[:])
                            ssq = ps.tile([1, 512], dt.float32, tag="s", bufs=4)
                            nc.tensor.matmul(ssq[:], ones_sb[:], q2[:],
                                             start=True, stop=True)
                            rms = work.tile([1, 512], dt.float32, tag="rms", bufs=2)
                            nc.scalar.activation(rms[:], ssq[:], AF.Sqrt,
                                                 bias=eps_sb[0:1, :],
                                                 scale=1.0 / HD)
                            rs = work.tile([1, 512], dt.float32, tag="rs", bufs=2)
                            nc.vector.reciprocal_approx_fast(rs[:], rms[:])
                            rsd = dpool.tile([1, 512], dt.float32, tag="rsd",
                                             bufs=4)
                            nc.gpsimd.dma_start(rsd[:], rs[:])
                            rsb = work.tile([P, 512], dt.float32, tag="rsb",
                                            bufs=2)
                            nc.gpsimd.dma_start(rsb[:],
                                                rsd[:].to_broadcast((P, 512)))
                            qs = work.tile([P, 512], dt.bfloat16, tag="qs", bufs=3)
                            nc.vector.scalar_tensor_tensor(
                                qs[:], qraw[:], wnorm[:], rsb[:],
                                op0=OP.mult, op1=OP.mult)
                            qsw = work.tile([P, 512], dt.bfloat16, tag="qsw",
                                            bufs=3)
                            nc.gpsimd.dma_start(qsw[0:64, :], qs[64:128, :])
                            nc.gpsimd.dma_start(qsw[64:128, :], qs[0:64, :])
                            t1 = work.tile([P, 512], dt.bfloat16, tag="t1", bufs=2)
                            nc.vector.tensor_mul(t1[:], qs[:], cos_sb[:, tsl])
                            t2 = work.tile([P, 512], dt.bfloat16, tag="t2", bufs=2)
                            nc.vector.tensor_mul(t2[:], qsw[:], sin_sb[:, tsl])
                            nc.vector.tensor_add(dest[:, tsl], t1[:], t2[:])

                    # ---- v projection, two subtiles at a time ----
                    for grp in range(2):
                        pv2 = [ps.tile([P, 256], dt.float32, tag="acc", bufs=4,
                                       name=f"pv{grp}_{i}") for i in range(2)]
                        for cc in range(CCH):
                            for i in range(2):
                                ts_ = grp * 2 + i
                                nc.tensor.matmul(
                                    pv2[i][:],
                                    xtc[:, cc, ts_ * 128:(ts_ + 1) * 128],
                                    wv_sb[:, cc, :],
                                    start=(cc == 0), stop=(cc == CCH - 1))
                        for i in range(2):
                            ts_ = grp * 2 + i
                            nc.vector.tensor_copy(vv[:, tp * 4 + ts_, :],
                                                  pv2[i][:])

                pair_state[p] = (qr_t, kr_t, vv)

            def emit_attn(p):
                qr_t, kr_t, vv = pair_state.pop(p)
                o_pair = [opool.tile([P, L], dt.bfloat16, tag="o", bufs=NHL,
                                     name=f"o{p}_{i}") for i in range(2)]
                o_tiles.extend(o_pair)
                for qp in range(TP):
                    qsl = bass.ts(qp, 512)
                    po = [ps.tile([P, 512], dt.float32, tag="acc", bufs=4,
                                  name=f"po{i}") for i in range(2)]
                    prs = [ps.tile([1, 512], dt.float32, tag="s", bufs=4,
                                   name=f"prs{i}") for i in range(2)]
                    for kc in range(KC):
                        for l in range(2):
                            pss = ps.tile([P, 512], dt.float32, tag="s", bufs=4)
                            nc.tensor.matmul(
                                pss[:], kr_t[l][:, kc * P:(kc + 1) * P],
                                qr_t[l][:, qsl], start=True, stop=True)
                            e = work.tile([P, 512], dt.bfloat16, tag="e", bufs=4)
                            nc.scalar.activation(e[:], pss[:], AF.Exp,
                                                 scale=scale)
                            nc.tensor.matmul(
                                po[l][:], vv[:, kc, l * 128:(l + 1) * 128], e[:],
                                start=(kc == 0), stop=(kc == KC - 1))
                            nc.tensor.matmul(
                                prs[l][:], ones_sb[:], e[:],
                                start=(kc == 0), stop=(kc == KC - 1))
                    for l in range(2):
                        rr = work.tile([1, 512], dt.float32, tag="rs", bufs=2)
                        nc.vector.reciprocal_approx_fast(rr[:], prs[l][:])
                        rrd = dpool.tile([1, 512], dt.float32, tag="rsd", bufs=4)
                        nc.gpsimd.dma_start(rrd[:], rr[:])
                        rrb = work.tile([P, 512], dt.float32, tag="rsb", bufs=2)
                        nc.gpsimd.dma_start(rrb[:], rrd[:].to_broadcast((P, 512)))
                        nc.vector.tensor_mul(o_pair[l][:, qsl], po[l][:], rrb[:])

            # software-pipeline: proj(p) overlaps attn(p-1)
            for p in range(NP):
                emit_proj(p)
                if p > 0:
                    emit_attn(p - 1)
            emit_attn(NP - 1)

            # ---------------- wo ----------------
            for jp in range(JP):
                jsl = bass.ts(jp, 512)
                wo_jp = stream.tile([P, NHL, 512], dt.bfloat16, tag="wo", bufs=2)
                wosrc = woT_d[:, jsl].rearrange("(h q) j -> q h j", q=P)
                for sp in range(4):
                    h0, h1 = sp * NHL // 4, (sp + 1) * NHL // 4
                    nc.sync.dma_start(wo_jp[:, h0:h1, :], wosrc[:, h0:h1, :])
                for tt in range(L // P):
                    pw = ps.tile([P, 512], dt.float32, tag="acc", bufs=4)
                    for h in range(NHL):
                        nc.tensor.matmul(
                            pw[:], o_tiles[h][:, tt * P:(tt + 1) * P],
                            wo_jp[:, h, :], start=(h == 0), stop=(h == NHL - 1))
                    osb = work.tile([P, 512], dt.float32, tag="outsb", bufs=2)
                    nc.vector.tensor_copy(osb[:], pw[:])
                    nc.sync.dma_start(out_d[tt * P:(tt + 1) * P, jsl], osb[:])

    nc.compile()
    return nc


def _host_prepare(x, rope_cos, rope_sin, wqkv, wo, q_norm_w, k_norm_w,
                  L, C, NP, DOUT, n_cores):
    """Build per-core input dicts."""
    NH_TOT = wqkv.shape[0] // 3 // HD
    NHL = 2 * NP
    n_seq = n_cores // 2
    perm = np.concatenate([np.arange(0, HD, 2), np.arange(1, HD, 2)])  # deinterleave

    qn_p = np.ascontiguousarray(q_norm_w[perm].reshape(HD, 1)).astype(np.float32)
    kn_p = np.ascontiguousarray(k_norm_w[perm].reshape(HD, 1)).astype(np.float32)

    wq = wqkv[0 * NH_TOT * HD:1 * NH_TOT * HD].reshape(NH_TOT, HD, C)
    wk = wqkv[1 * NH_TOT * HD:2 * NH_TOT * HD].reshape(NH_TOT, HD, C)
    wv = wqkv[2 * NH_TOT * HD:3 * NH_TOT * HD].reshape(NH_TOT, HD, C)

    in_maps = []
    for c in range(n_cores):
        b = c // 2
        hg = c % 2
        heads = list(range(hg * NHL, hg * NHL + NHL))
        xb = x[b * L:(b + 1) * L]                      # [L, C]
        xT = np.ascontiguousarray(xb.T).astype(BF)      # [C, L]

        blocks = []
        for pidx in range(NP):
            h0, h1 = heads[2 * pidx], heads[2 * pidx + 1]
            blocks += [wq[h0][perm], wq[h1][perm],
                       wk[h0][perm], wk[h1][perm],
                       wv[h0], wv[h1]]
        wT = np.ascontiguousarray(np.concatenate(blocks, axis=0).T).astype(BF)

        woT_rows = wo[:, heads[0] * HD:(heads[-1] + 1) * HD].T  # [NHL*HD, DOUT]
        woT = np.ascontiguousarray(woT_rows).astype(BF)

        cosb = rope_cos[b * L:(b + 1) * L].T            # [64, L]
        sinb = rope_sin[b * L:(b + 1) * L].T
        cosT = np.ascontiguousarray(np.concatenate([cosb, cosb], 0)).astype(BF)
        sinT = np.ascontiguousarray(np.concatenate([-sinb, sinb], 0)).astype(BF)

        in_maps.append({
            "xT": xT, "wT": wT, "woT": woT, "cosT": cosT, "sinT": sinT,
            "qnw": qn_p, "knw": kn_p,
        })
    return in_maps


def _reference_numpy(x, rope_cos, rope_sin, cu, max_length,
                     wqkv, wo, q_norm_w, k_norm_w):
    """Pure-numpy fallback (exact reference math) for non-uniform cu."""
    T, dim = x.shape
    nh = dim // HD
    qkv = (x @ wqkv.T).reshape(T, 3, nh, HD)
    q, k, v = qkv[:, 0], qkv[:, 1], qkv[:, 2]

    def rmsnorm(t, w):
        return t / np.sqrt((t * t).mean(-1, keepdims=True) + 1e-5) * w

    def rope(t):
        tr = t.reshape(t.shape[:-1] + (HD // 2, 2))
        e, o = tr[..., 0], tr[..., 1]
        cc = rope_cos[:, None, :]
        ss = rope_sin[:, None, :]
        return np.stack([e * cc - o * ss, e * ss + o * cc], -1).reshape(t.shape)

    q = rope(rmsnorm(q, q_norm_w))
    k = rope(rmsnorm(k, k_norm_w))
    o = np.zeros((T, nh, HD), np.float32)
    nb = len(cu) - 1
    for i in range(nb):
        s, e_ = int(cu[i]), int(cu[i + 1])
        if e_ <= s:
            continue
        qs_, ks_, vs_ = q[s:e_], k[s:e_], v[s:e_]
        sc = np.einsum("lhd,mhd->hlm", qs_, ks_) / math.sqrt(HD)
        sc = sc - sc.max(-1, keepdims=True)
        a = np.exp(sc)
        a /= a.sum(-1, keepdims=True)
        o[s:e_] = np.einsum("hlm,mhd->lhd", a, vs_)
    return (o.reshape(T, dim) @ wo.T).astype(np.float32)


def kernel(x, rope_cos, rope_sin, cu, max_length, wqkv, wo, q_norm_w, k_norm_w):
    x = np.asarray(x, np.float32)
    rope_cos = np.asarray(rope_cos, np.float32)
    rope_sin = np.asarray(rope_sin, np.float32)
    cu = np.asarray(cu)
    wqkv = np.asarray(wqkv, np.float32)
    wo = np.asarray(wo, np.float32)
    q_norm_w = np.asarray(q_norm_w, np.float32)
    k_norm_w = np.asarray(k_norm_w, np.float32)

    T, C = x.shape
    N_CORES = 8
    L = T // 4
    expect_cu = np.arange(5) * L
    if (len(cu) != 5 or not np.array_equal(np.asarray(cu).ravel(), expect_cu)
            or T % 4 != 0 or L % 512 != 0 or C % P != 0):
        return _reference_numpy(x, rope_cos, rope_sin, cu, max_length,
                                wqkv, wo, q_norm_w, k_norm_w)

    NP = (C // HD) // 2 // 2          # local head pairs = NH/2/2
    DOUT = wo.shape[0]

    from concourse.bass_utils import run_bass_kernel_spmd

    nc = _build_program(L, C, NP, DOUT, N_CORES)
    in_maps = _host_prepare(x, rope_cos, rope_sin, wqkv, wo, q_norm_w, k_norm_w,
                            L, C, NP, DOUT, N_CORES)
    res = run_bass_kernel_spmd(nc, in_maps, list(range(N_CORES)))

    out = np.empty((T, DOUT), np.float32)
    for b in range(4):
        out[b * L:(b + 1) * L] = (res.results[2 * b]["out"]
                                  + res.results[2 * b + 1]["out"])
    return out
